# revision 10
# baseline (speedup 1.0000x reference)
"""Trainium2 Bass kernel for nn_BlockPiecewiseLinear (histogram_binning).

Math (validated vs the JAX reference):
    S     = softplus(slope)  in fp16 (ACT: exp + ln(1+x))
    cnt   = sum_j 1[x_j <= q]  (UNSORTED x, fp32-exact compare)
    ssel  = S[cnt] + EPS   via  sum_j 1[cnt==j] * S_j   (one-hot: exact mask,
            only fp16 value rounding)
    xs    = sort(x) in fp16 (bitonic, 15 layers, all unit-stride -> DVE 2x)
    M_j   = 1[cnt >= j]
    W     = sum_j M_j * (S_j - S_{j-1}) * xs[j-1]        (fp16 chain)
    out   = q*ssel - xs[0]*(S_0+EPS-1) - W + y_bias

All compute on DVE+ACT.  GPSIMD is used only as a third DMA queue: running
Q7 compute concurrently with DVE steals the shared SBUF port and slows DVE
~2x (measured), so no compute is placed there.
Layout [P, K, G]: knot dim middle, G=128 rows innermost -> every op
unit-stride (fp16 2x mode); cnt path reads x in fp32 for exactness.
Sharding: pure data-parallel over batch across 8 NeuronCores.
"""

import numpy as np

import concourse.bass as bass
import concourse.bacc as bacc
import concourse.mybir as mybir
import concourse.tile as tile
from concourse.bass_utils import run_bass_kernel_spmd

F32 = mybir.dt.float32
F16 = mybir.dt.float16
Alu = mybir.AluOpType
Act = mybir.ActivationFunctionType
AxX = mybir.AxisListType.X

B, F, K = 4096, 512, 32
KP1 = K + 1
EPS = 1e-3
NCORES = 8
P = 128
G = 128  # rows (innermost) per tile per partition


def _bitonic_layers(n=32):
    layers = []
    k = 2
    while k <= n:
        j = k // 2
        while j >= 1:
            layers.append((k, j))
            j //= 2
        k *= 2
    return layers  # 15 layers for n=32


def _ap(t_ap, off_elems, dims):
    # count-1 free dims disqualify the DVE 2x perf mode; drop them
    dims = [e for e in dims if e[1] != 1]
    return bass.AP(tensor=t_ap.tensor, offset=t_ap.offset + off_elems,
                   ap=[t_ap.ap[0]] + dims)


def _emit_sort_layer(nc, cur, dst, kk, jj, g):
    """One bitonic layer in [P, K, G] layout (knot middle, rows inner)."""
    cs = K // (2 * kk) if kk < K else 0
    base_s = cur[:, :, :]
    base_d = dst[:, :, :]
    inner = [1, jj * g]
    if kk < K and jj == kk // 2:
        dims_in = [[2 * kk * g, cs], [kk * g, 2], inner]
        in_lo = _ap(base_s, 0, dims_in)
        in_hi = _ap(base_s, jj * g, dims_in)
        out_min = _ap(base_d, 0, [[2 * kk * g, cs], [(kk + jj) * g, 2], inner])
        out_max = _ap(base_d, jj * g, [[2 * kk * g, cs], [(kk - jj) * g, 2], inner])
        nc.vector.tensor_tensor(out=out_min, in0=in_lo, in1=in_hi, op=Alu.min)
        nc.vector.tensor_tensor(out=out_max, in0=in_lo, in1=in_hi, op=Alu.max)
    elif kk == K:
        ms = K // (2 * jj)
        dims_in = [[2 * jj * g, ms], inner]
        in_lo = _ap(base_s, 0, dims_in)
        in_hi = _ap(base_s, jj * g, dims_in)
        nc.vector.tensor_tensor(out=_ap(base_d, 0, dims_in), in0=in_lo, in1=in_hi, op=Alu.min)
        nc.vector.tensor_tensor(out=_ap(base_d, jj * g, dims_in), in0=in_lo, in1=in_hi, op=Alu.max)
    elif cs == 1:
        ms = kk // (2 * jj)
        dims_in = [[kk * g, 2], [2 * jj * g, ms], inner]
        in_lo = _ap(base_s, 0, dims_in)
        in_hi = _ap(base_s, jj * g, dims_in)
        out_min = _ap(base_d, 0, [[(kk + jj) * g, 2], [2 * jj * g, ms], inner])
        out_max = _ap(base_d, jj * g, [[(kk - jj) * g, 2], [2 * jj * g, ms], inner])
        nc.vector.tensor_tensor(out=out_min, in0=in_lo, in1=in_hi, op=Alu.min)
        nc.vector.tensor_tensor(out=out_max, in0=in_lo, in1=in_hi, op=Alu.max)
    else:
        ms = kk // (2 * jj)
        for d in range(2):
            off = d * kk * g
            dims_in = [[2 * kk * g, cs], [2 * jj * g, ms], inner]
            in_lo = _ap(base_s, off, dims_in)
            in_hi = _ap(base_s, off + jj * g, dims_in)
            o_lo = _ap(base_d, off, dims_in)
            o_hi = _ap(base_d, off + jj * g, dims_in)
            op_lo, op_hi = (Alu.min, Alu.max) if d == 0 else (Alu.max, Alu.min)
            nc.vector.tensor_tensor(out=o_lo, in0=in_lo, in1=in_hi, op=op_lo)
            nc.vector.tensor_tensor(out=o_hi, in0=in_lo, in1=in_hi, op=op_hi)


def build_nc(nloc, g=G):
    rows_per_tile = P * g
    T = nloc // rows_per_tile
    assert T * rows_per_tile == nloc
    TG = T * g

    nc = bacc.Bacc("TRN2", target_bir_lowering=False, debug=False)
    x_d = nc.declare_dram_parameter("x", [T * P, K * g], F32, isOutput=False)
    sl_d = nc.declare_dram_parameter("sl", [T * P, KP1 * g], F16, isOutput=False)
    q_d = nc.declare_dram_parameter("q", [P, TG], F32, isOutput=False)
    io_d = nc.declare_dram_parameter("iota", [P, KP1 * g], F16, isOutput=False)
    yb_d = nc.declare_dram_parameter("yb", [P, g], F32, isOutput=False)
    out_d = nc.declare_dram_parameter("out", [P, TG], F32, isOutput=True)
    ss_d = nc.declare_dram_parameter("ssel", [P, TG], F32, isOutput=True)

    xv = x_d[:, :].rearrange("(t p) f -> t p f", p=P)
    slv = sl_d[:, :].rearrange("(t p) f -> t p f", p=P)

    layers = _bitonic_layers(K)

    with tile.TileContext(nc) as tc:
        with (
            tc.tile_pool(name="pacc", bufs=1) as pacc,
            tc.tile_pool(name="pxf", bufs=2) as pxf,
            tc.tile_pool(name="pxh", bufs=2) as pxh,
            tc.tile_pool(name="psort", bufs=2) as psort,
            tc.tile_pool(name="psl", bufs=2) as psl,
            tc.tile_pool(name="pSh", bufs=2) as pSh,
            tc.tile_pool(name="p16", bufs=1) as p16,
        ):
            q_t = pacc.tile([P, T, g], F32, tag="q")
            nc.gpsimd.dma_start(out=q_t[:, 0:T // 2, :], in_=q_d[:, 0:TG // 2])
            iota_t = pacc.tile([P, KP1, g], F16, tag="iota")
            nc.scalar.dma_start(out=iota_t[:, :, :], in_=io_d[:, :])
            yb_t = pacc.tile([P, g], F32, tag="yb")
            nc.scalar.dma_start(out=yb_t[:, :], in_=yb_d[:, :])
            W_t = pacc.tile([P, T, g], F32, tag="W")
            ss_t = pacc.tile([P, T, g], F32, tag="ss")
            v_t = pacc.tile([P, T, g], F32, tag="v")

            def epilogue(lo, hi):
                # in-place on q_t (its slices are consumed by the tiles' stepU
                # before each half runs); out = q*ssel - v - W + yb  where
                # v = xs0*(S0+EPS-1) and ssel already includes +EPS.
                def s(t3):
                    return t3[:, lo:hi, :]
                nc.vector.tensor_tensor(out=s(q_t), in0=s(q_t), in1=s(ss_t), op=Alu.mult)
                nc.vector.tensor_tensor(out=s(q_t), in0=s(q_t), in1=s(v_t), op=Alu.subtract)
                nc.vector.tensor_tensor(out=s(q_t), in0=s(q_t), in1=s(W_t), op=Alu.subtract)
                ybb = bass.AP(tensor=yb_t[:, :].tensor, offset=yb_t[:, :].offset,
                              ap=[yb_t[:, :].ap[0], [0, hi - lo], [1, g]])
                nc.vector.tensor_tensor(out=s(q_t), in0=s(q_t), in1=ybb, op=Alu.add)
                nc.scalar.dma_start(out=out_d[:, lo * g:hi * g], in_=q_t[:, lo:hi, :])
                nc.scalar.dma_start(out=ss_d[:, lo * g:hi * g], in_=ss_t[:, lo:hi, :])

            for t in range(T):
                x_f = pxf.tile([P, K, g], F32, tag="x")
                if t == 0:
                    # split the first x tile across both queues to halve ramp
                    half = K * g // 2
                    nc.sync.dma_start(out=x_f[:, 0:K // 2, :], in_=xv[0][:, 0:half])
                    nc.gpsimd.dma_start(out=x_f[:, K // 2:K, :], in_=xv[0][:, half:])
                elif t % 2 == 0:
                    nc.sync.dma_start(out=x_f[:, :, :], in_=xv[t])
                else:
                    nc.gpsimd.dma_start(out=x_f[:, :, :], in_=xv[t])
                sl_t = psl.tile([P, KP1, g], F16, tag="sl")
                nc.scalar.dma_start(out=sl_t[:, :, :], in_=slv[t])
                if t == 1:
                    nc.gpsimd.dma_start(out=q_t[:, T // 2:T, :], in_=q_d[:, TG // 2:TG])

                # ---- ACT: softplus -> fp16, x cast -> fp16 ----
                Sh_t = pSh.tile([P, KP1, g], F16, tag="Sh")
                nc.scalar.activation(out=sl_t[:, :, :], in_=sl_t[:, :, :], func=Act.Exp)
                nc.scalar.activation(out=Sh_t[:, :, :], in_=sl_t[:, :, :], func=Act.Ln, bias=1.0)
                xh_t = pxh.tile([P, K, g], F16, tag="xh")
                nc.scalar.activation(out=xh_t[:, :, :], in_=x_f[:, :, :], func=Act.Copy)

                # ---- cnt: fp32-exact compare, transposed so the reduce is
                # innermost over K (fp16 2x reduce; counts <=32 exact) ----
                xf = x_f[:, :, :]
                x_T = bass.AP(tensor=xf.tensor, offset=xf.offset,
                              ap=[xf.ap[0], [1, g], [g, K]])
                q2 = q_t[:, t, :]
                q_T = bass.AP(tensor=q2.tensor, offset=q2.offset,
                              ap=[q2.ap[0], [1, g], [0, K]])
                stepT = p16.tile([P, g, K], F16, tag="su")
                nc.vector.tensor_tensor(out=stepT[:, :, :], in0=x_T, in1=q_T, op=Alu.is_le)
                cnt = p16.tile([P, g], F16, tag="cnt")
                with nc.allow_low_precision(reason="cnt <= 32 is exact in fp16"):
                    nc.vector.tensor_reduce(out=cnt[:, :], in_=stepT[:, :, :], axis=AxX, op=Alu.add)

                # ---- fp16 bitonic sort ----
                cur = xh_t
                for kk, jj in layers:
                    dst = psort.tile([P, K, g], F16, tag="sort")
                    _emit_sort_layer(nc, cur, dst, kk, jj, g)
                    cur = dst
                xs_t = cur

                # ---- masks from cnt (emitted after the sort so the iota DMA
                # is never on the tile-0 critical path) ----
                cnt2 = cnt[:, :]
                cb_kp1 = bass.AP(tensor=cnt2.tensor, offset=cnt2.offset,
                                 ap=[cnt2.ap[0], [0, KP1], [1, g]])
                delta = p16.tile([P, KP1, g], F16, tag="delta")
                nc.vector.tensor_tensor(out=delta[:, :, :], in0=cb_kp1, in1=iota_t[:, :, :], op=Alu.is_equal)
                cb_k = bass.AP(tensor=cnt2.tensor, offset=cnt2.offset,
                               ap=[cnt2.ap[0], [0, K], [1, g]])
                M_t = p16.tile([P, K, g], F16, tag="M")
                nc.vector.tensor_tensor(out=M_t[:, :, :], in0=cb_k, in1=iota_t[:, 1:KP1, :], op=Alu.is_ge)

                # ---- ssel: one-hot gather of fp16 S at cnt ----
                sselP = p16.tile([P, KP1, g], F16, tag="sselP")
                nc.vector.tensor_tensor(out=sselP[:, :, :], in0=delta[:, :, :], in1=Sh_t[:, :, :], op=Alu.mult)
                s1 = p16.tile([P, 16, g], F16, tag="c1")
                nc.vector.tensor_tensor(out=s1[:, :, :], in0=sselP[:, 0:16, :], in1=sselP[:, 16:32, :], op=Alu.add)
                s2 = p16.tile([P, 8, g], F16, tag="c2")
                nc.vector.tensor_tensor(out=s2[:, :, :], in0=s1[:, 0:8, :], in1=s1[:, 8:16, :], op=Alu.add)
                s3 = p16.tile([P, 4, g], F16, tag="c3")
                nc.vector.tensor_tensor(out=s3[:, :, :], in0=s2[:, 0:4, :], in1=s2[:, 4:8, :], op=Alu.add)
                s4 = p16.tile([P, 2, g], F16, tag="c4")
                nc.vector.tensor_tensor(out=s4[:, :, :], in0=s3[:, 0:2, :], in1=s3[:, 2:4, :], op=Alu.add)
                s5 = p16.tile([P, 1, g], F16, tag="cnt2")
                nc.vector.tensor_tensor(out=s5[:, :, :], in0=s4[:, 0:1, :], in1=s4[:, 1:2, :], op=Alu.add)
                # ssel = s5 + EPS + sselP[K]  (fused; ss_t then holds final ssel)
                nc.vector.scalar_tensor_tensor(out=ss_t[:, t, :], in0=s5[:, 0, :],
                                               scalar=EPS, in1=sselP[:, K, :],
                                               op0=Alu.add, op1=Alu.add)

                # ---- W chain (fp16) ----
                dS = p16.tile([P, K, g], F16, tag="dS")
                nc.vector.tensor_tensor(out=dS[:, :, :], in0=Sh_t[:, 1:KP1, :], in1=Sh_t[:, 0:K, :], op=Alu.subtract)
                m_t = p16.tile([P, K, g], F16, tag="m")
                nc.vector.tensor_tensor(out=m_t[:, :, :], in0=M_t[:, :, :], in1=dS[:, :, :], op=Alu.mult)
                w_t = p16.tile([P, K, g], F16, tag="delta")
                nc.vector.tensor_tensor(out=w_t[:, :, :], in0=m_t[:, :, :], in1=xs_t[:, :, :], op=Alu.mult)
                t1 = p16.tile([P, 16, g], F16, tag="c1")
                nc.vector.tensor_tensor(out=t1[:, :, :], in0=w_t[:, 0:16, :], in1=w_t[:, 16:32, :], op=Alu.add)
                t2 = p16.tile([P, 8, g], F16, tag="c2")
                nc.vector.tensor_tensor(out=t2[:, :, :], in0=t1[:, 0:8, :], in1=t1[:, 8:16, :], op=Alu.add)
                t3 = p16.tile([P, 4, g], F16, tag="c3")
                nc.vector.tensor_tensor(out=t3[:, :, :], in0=t2[:, 0:4, :], in1=t2[:, 4:8, :], op=Alu.add)
                t4 = p16.tile([P, 2, g], F16, tag="c4")
                nc.vector.tensor_tensor(out=t4[:, :, :], in0=t3[:, 0:2, :], in1=t3[:, 2:4, :], op=Alu.add)
                nc.vector.tensor_tensor(out=W_t[:, t, :], in0=t4[:, 0, :], in1=t4[:, 1, :], op=Alu.add)

                # ---- v = xs0 * (S0 + EPS - 1)  (fused epilogue input) ----
                nc.vector.scalar_tensor_tensor(out=v_t[:, t, :], in0=Sh_t[:, 0, :],
                                               scalar=EPS - 1.0, in1=xs_t[:, 0, :],
                                               op0=Alu.add, op1=Alu.mult)

                if t == T // 2:
                    epilogue(0, T // 2)
            epilogue(T // 2, T)
    nc.compile()
    return nc


_NC_CACHE = {}


def _get_nc(nloc, g=G):
    key = (nloc, g)
    if key not in _NC_CACHE:
        _NC_CACHE[key] = build_nc(nloc, g)
    return _NC_CACHE[key]


def prepare_in_maps(inputs, x_pos, slope, y_bias):
    inputs = np.asarray(inputs, dtype=np.float32)
    x_pos = np.asarray(x_pos, dtype=np.float32)
    slope = np.asarray(slope, dtype=np.float32)
    y_bias = np.asarray(y_bias, dtype=np.float32)
    b, f = inputs.shape
    bloc = b // NCORES
    nloc = bloc * f
    T = nloc // (P * G)

    iota = np.arange(KP1, dtype=np.float16)
    iota_full = np.ascontiguousarray(
        np.broadcast_to(iota[None, :, None], (P, KP1, G)).reshape(P, KP1 * G)
    )
    pg_idx = (np.arange(P)[:, None] * G + np.arange(G)[None, :]) % f
    yb_exp = np.ascontiguousarray(y_bias[:, 0][pg_idx])

    in_maps = []
    for c in range(NCORES):
        sl_b = slice(c * bloc, (c + 1) * bloc)
        x = x_pos[sl_b].reshape(T, P, G, K).transpose(0, 1, 3, 2)
        x = np.ascontiguousarray(x).reshape(T * P, K * G)
        sl = slope[sl_b].astype(np.float16).reshape(T, P, G, KP1).transpose(0, 1, 3, 2)
        sl = np.ascontiguousarray(sl).reshape(T * P, KP1 * G)
        q = inputs[sl_b].reshape(T, P, G).transpose(1, 0, 2)
        q = np.ascontiguousarray(q).reshape(P, T * G)
        in_maps.append({"x": x, "sl": sl, "q": q, "iota": iota_full, "yb": yb_exp})
    return nloc, in_maps


def unpack_results(res, b, f):
    bloc = b // NCORES
    nloc = bloc * f
    T = nloc // (P * G)
    outs, ssels = [], []
    for c in range(NCORES):
        o = res.results[c]["out"].reshape(P, T, G).transpose(1, 0, 2).reshape(bloc, f)
        s = res.results[c]["ssel"].reshape(P, T, G).transpose(1, 0, 2).reshape(bloc, f)
        outs.append(o)
        ssels.append(s)
    return np.concatenate(outs, 0), np.concatenate(ssels, 0)


def kernel(inputs, x_pos, slope, y_bias):
    b, f = np.asarray(inputs).shape
    nloc, in_maps = prepare_in_maps(inputs, x_pos, slope, y_bias)
    nc = _get_nc(nloc)
    res = run_bass_kernel_spmd(nc, in_maps, list(range(NCORES)))
    return unpack_results(res, b, f)


# revision 12
# speedup vs baseline: 1.0692x; 1.0692x over previous
"""Trainium2 Bass kernel for nn_BlockPiecewiseLinear (histogram_binning).

Math (validated vs the JAX reference):
    S     = softplus(slope)  in fp16 (ACT: exp + ln(1+x))
    cnt   = sum_j 1[x_j <= q]  (UNSORTED x, fp32-exact compare)
    ssel  = S[cnt] + EPS   via  sum_j 1[cnt==j] * S_j   (one-hot: exact mask,
            only fp16 value rounding)
    xs    = sort(x) in fp16 (bitonic, 15 layers, all unit-stride -> DVE 2x)
    M_j   = 1[cnt >= j]
    W     = sum_j M_j * (S_j - S_{j-1}) * xs[j-1]        (fp16 chain)
    out   = q*ssel - xs[0]*(S_0+EPS-1) - W + y_bias

All compute on DVE+ACT.  GPSIMD is used only as a third DMA queue: running
Q7 compute concurrently with DVE steals the shared SBUF port and slows DVE
~2x (measured), so no compute is placed there.
Layout [P, K, G]: knot dim middle, G=128 rows innermost -> every op
unit-stride (fp16 2x mode); cnt path reads x in fp32 for exactness.
Sharding: pure data-parallel over batch across 8 NeuronCores.
"""

import numpy as np

import concourse.bass as bass
import concourse.bacc as bacc
import concourse.mybir as mybir
import concourse.tile as tile
from concourse.bass_utils import run_bass_kernel_spmd

F32 = mybir.dt.float32
F16 = mybir.dt.float16
Alu = mybir.AluOpType
Act = mybir.ActivationFunctionType
AxX = mybir.AxisListType.X

B, F, K = 4096, 512, 32
KP1 = K + 1
EPS = 1e-3
NCORES = 8
P = 128
G = 128  # rows (innermost) per tile per partition


def _bitonic_layers(n=32):
    layers = []
    k = 2
    while k <= n:
        j = k // 2
        while j >= 1:
            layers.append((k, j))
            j //= 2
        k *= 2
    return layers  # 15 layers for n=32


def _ap(t_ap, off_elems, dims):
    # count-1 free dims disqualify the DVE 2x perf mode; drop them
    dims = [e for e in dims if e[1] != 1]
    return bass.AP(tensor=t_ap.tensor, offset=t_ap.offset + off_elems,
                   ap=[t_ap.ap[0]] + dims)


def _emit_sort_layer(nc, cur, dst, kk, jj, g):
    """One bitonic layer in [P, K, G] layout (knot middle, rows inner)."""
    cs = K // (2 * kk) if kk < K else 0
    base_s = cur[:, :, :]
    base_d = dst[:, :, :]
    inner = [1, jj * g]
    if kk < K and jj == kk // 2:
        dims_in = [[2 * kk * g, cs], [kk * g, 2], inner]
        in_lo = _ap(base_s, 0, dims_in)
        in_hi = _ap(base_s, jj * g, dims_in)
        out_min = _ap(base_d, 0, [[2 * kk * g, cs], [(kk + jj) * g, 2], inner])
        out_max = _ap(base_d, jj * g, [[2 * kk * g, cs], [(kk - jj) * g, 2], inner])
        nc.vector.tensor_tensor(out=out_min, in0=in_lo, in1=in_hi, op=Alu.min)
        nc.vector.tensor_tensor(out=out_max, in0=in_lo, in1=in_hi, op=Alu.max)
    elif kk == K:
        ms = K // (2 * jj)
        dims_in = [[2 * jj * g, ms], inner]
        in_lo = _ap(base_s, 0, dims_in)
        in_hi = _ap(base_s, jj * g, dims_in)
        nc.vector.tensor_tensor(out=_ap(base_d, 0, dims_in), in0=in_lo, in1=in_hi, op=Alu.min)
        nc.vector.tensor_tensor(out=_ap(base_d, jj * g, dims_in), in0=in_lo, in1=in_hi, op=Alu.max)
    elif cs == 1:
        ms = kk // (2 * jj)
        dims_in = [[kk * g, 2], [2 * jj * g, ms], inner]
        in_lo = _ap(base_s, 0, dims_in)
        in_hi = _ap(base_s, jj * g, dims_in)
        out_min = _ap(base_d, 0, [[(kk + jj) * g, 2], [2 * jj * g, ms], inner])
        out_max = _ap(base_d, jj * g, [[(kk - jj) * g, 2], [2 * jj * g, ms], inner])
        nc.vector.tensor_tensor(out=out_min, in0=in_lo, in1=in_hi, op=Alu.min)
        nc.vector.tensor_tensor(out=out_max, in0=in_lo, in1=in_hi, op=Alu.max)
    else:
        ms = kk // (2 * jj)
        for d in range(2):
            off = d * kk * g
            dims_in = [[2 * kk * g, cs], [2 * jj * g, ms], inner]
            in_lo = _ap(base_s, off, dims_in)
            in_hi = _ap(base_s, off + jj * g, dims_in)
            o_lo = _ap(base_d, off, dims_in)
            o_hi = _ap(base_d, off + jj * g, dims_in)
            op_lo, op_hi = (Alu.min, Alu.max) if d == 0 else (Alu.max, Alu.min)
            nc.vector.tensor_tensor(out=o_lo, in0=in_lo, in1=in_hi, op=op_lo)
            nc.vector.tensor_tensor(out=o_hi, in0=in_lo, in1=in_hi, op=op_hi)


def build_nc(nloc, g=G):
    rows_per_tile = P * g
    T = nloc // rows_per_tile
    assert T * rows_per_tile == nloc
    TG = T * g

    nc = bacc.Bacc("TRN2", target_bir_lowering=False, debug=False)
    x_d = nc.declare_dram_parameter("x", [T * P, K * g], F32, isOutput=False)
    sl_d = nc.declare_dram_parameter("sl", [T * P, KP1 * g], F16, isOutput=False)
    q_d = nc.declare_dram_parameter("q", [P, TG], F32, isOutput=False)
    io_d = nc.declare_dram_parameter("iota", [P, KP1 * g], F16, isOutput=False)
    yb_d = nc.declare_dram_parameter("yb", [P, g], F32, isOutput=False)
    out_d = nc.declare_dram_parameter("out", [P, TG], F32, isOutput=True)
    ss_d = nc.declare_dram_parameter("ssel", [P, TG], F32, isOutput=True)

    xv = x_d[:, :].rearrange("(t p) f -> t p f", p=P)
    slv = sl_d[:, :].rearrange("(t p) f -> t p f", p=P)

    layers = _bitonic_layers(K)

    with tile.TileContext(nc) as tc:
        with (
            tc.tile_pool(name="pacc", bufs=1) as pacc,
            tc.tile_pool(name="pxf", bufs=2) as pxf,
            tc.tile_pool(name="pxh", bufs=2) as pxh,
            tc.tile_pool(name="psort", bufs=2) as psort,
            tc.tile_pool(name="psl", bufs=2) as psl,
            tc.tile_pool(name="pSh", bufs=2) as pSh,
            tc.tile_pool(name="p16", bufs=1) as p16,
        ):
            q_t = pacc.tile([P, T, g], F32, tag="q")
            nc.gpsimd.dma_start(out=q_t[:, 0:T // 2, :], in_=q_d[:, 0:TG // 2])
            iota_t = pacc.tile([P, KP1, g], F16, tag="iota")
            nc.scalar.dma_start(out=iota_t[:, :, :], in_=io_d[:, :])
            yb_t = pacc.tile([P, g], F32, tag="yb")
            nc.scalar.dma_start(out=yb_t[:, :], in_=yb_d[:, :])
            W_t = pacc.tile([P, T, g], F32, tag="W")
            ss_t = pacc.tile([P, T, g], F32, tag="ss")
            v_t = pacc.tile([P, T, g], F32, tag="v")

            def epilogue(lo, hi):
                # in-place on q_t (its slices are consumed by the tiles' stepU
                # before each half runs); out = q*ssel - v - W + yb  where
                # v = xs0*(S0+EPS-1) and ssel already includes +EPS.
                def s(t3):
                    return t3[:, lo:hi, :]
                nc.vector.tensor_tensor(out=s(q_t), in0=s(q_t), in1=s(ss_t), op=Alu.mult)
                nc.vector.tensor_tensor(out=s(q_t), in0=s(q_t), in1=s(v_t), op=Alu.subtract)
                nc.vector.tensor_tensor(out=s(q_t), in0=s(q_t), in1=s(W_t), op=Alu.subtract)
                ybb = bass.AP(tensor=yb_t[:, :].tensor, offset=yb_t[:, :].offset,
                              ap=[yb_t[:, :].ap[0], [0, hi - lo], [1, g]])
                nc.vector.tensor_tensor(out=s(q_t), in0=s(q_t), in1=ybb, op=Alu.add)
                nc.scalar.dma_start(out=out_d[:, lo * g:hi * g], in_=q_t[:, lo:hi, :])
                nc.scalar.dma_start(out=ss_d[:, lo * g:hi * g], in_=ss_t[:, lo:hi, :])

            for t in range(T):
                x_f = pxf.tile([P, K, g], F32, tag="x")
                if t == 0:
                    # split the first x tile across both queues to halve ramp
                    half = K * g // 2
                    nc.sync.dma_start(out=x_f[:, 0:K // 2, :], in_=xv[0][:, 0:half])
                    nc.gpsimd.dma_start(out=x_f[:, K // 2:K, :], in_=xv[0][:, half:])
                elif t % 2 == 0:
                    nc.sync.dma_start(out=x_f[:, :, :], in_=xv[t])
                else:
                    nc.gpsimd.dma_start(out=x_f[:, :, :], in_=xv[t])
                sl_t = psl.tile([P, KP1, g], F16, tag="sl")
                nc.scalar.dma_start(out=sl_t[:, :, :], in_=slv[t])
                if t == 1:
                    nc.gpsimd.dma_start(out=q_t[:, T // 2:T, :], in_=q_d[:, TG // 2:TG])

                # ---- ACT: softplus -> fp16, x cast -> fp16 ----
                Sh_t = pSh.tile([P, KP1, g], F16, tag="Sh")
                nc.scalar.activation(out=sl_t[:, :, :], in_=sl_t[:, :, :], func=Act.Exp)
                nc.scalar.activation(out=Sh_t[:, :, :], in_=sl_t[:, :, :], func=Act.Ln, bias=1.0)
                xh_t = pxh.tile([P, K, g], F16, tag="xh")
                nc.scalar.activation(out=xh_t[:, :, :], in_=x_f[:, :, :], func=Act.Copy)

                # ---- cnt: fp32-exact compare + fp16 binary-tree count ----
                stepU = p16.tile([P, K, g], F16, tag="su")
                q2 = q_t[:, t, :]
                qb = bass.AP(tensor=q2.tensor, offset=q2.offset,
                             ap=[q2.ap[0], [0, K], [1, g]])
                nc.vector.tensor_tensor(out=stepU[:, :, :], in0=x_f[:, :, :], in1=qb, op=Alu.is_le)
                c1 = p16.tile([P, 16, g], F16, tag="c1")
                nc.vector.tensor_tensor(out=c1[:, :, :], in0=stepU[:, 0:16, :], in1=stepU[:, 16:32, :], op=Alu.add)
                c2 = p16.tile([P, 8, g], F16, tag="c2")
                nc.vector.tensor_tensor(out=c2[:, :, :], in0=c1[:, 0:8, :], in1=c1[:, 8:16, :], op=Alu.add)
                c3 = p16.tile([P, 4, g], F16, tag="c3")
                nc.vector.tensor_tensor(out=c3[:, :, :], in0=c2[:, 0:4, :], in1=c2[:, 4:8, :], op=Alu.add)
                c4 = p16.tile([P, 2, g], F16, tag="c4")
                nc.vector.tensor_tensor(out=c4[:, :, :], in0=c3[:, 0:2, :], in1=c3[:, 2:4, :], op=Alu.add)
                cntt = p16.tile([P, 1, g], F16, tag="cnt")
                nc.vector.tensor_tensor(out=cntt[:, :, :], in0=c4[:, 0:1, :], in1=c4[:, 1:2, :], op=Alu.add)
                cnt = cntt[:, 0, :]

                # ---- fp16 bitonic sort ----
                cur = xh_t
                for kk, jj in layers:
                    dst = psort.tile([P, K, g], F16, tag="sort")
                    _emit_sort_layer(nc, cur, dst, kk, jj, g)
                    cur = dst
                xs_t = cur

                # ---- masks from cnt (emitted after the sort so the iota DMA
                # is never on the tile-0 critical path) ----
                cnt2 = cnt
                cb_kp1 = bass.AP(tensor=cnt2.tensor, offset=cnt2.offset,
                                 ap=[cnt2.ap[0], [0, KP1], [1, g]])
                delta = p16.tile([P, KP1, g], F16, tag="delta")
                nc.vector.tensor_tensor(out=delta[:, :, :], in0=cb_kp1, in1=iota_t[:, :, :], op=Alu.is_equal)
                cb_k = bass.AP(tensor=cnt2.tensor, offset=cnt2.offset,
                               ap=[cnt2.ap[0], [0, K], [1, g]])
                M_t = p16.tile([P, K, g], F16, tag="M")
                nc.vector.tensor_tensor(out=M_t[:, :, :], in0=cb_k, in1=iota_t[:, 1:KP1, :], op=Alu.is_ge)

                # ---- ssel: one-hot gather of fp16 S at cnt ----
                sselP = p16.tile([P, KP1, g], F16, tag="sselP")
                nc.vector.tensor_tensor(out=sselP[:, :, :], in0=delta[:, :, :], in1=Sh_t[:, :, :], op=Alu.mult)
                s1 = p16.tile([P, 16, g], F16, tag="c1")
                nc.vector.tensor_tensor(out=s1[:, :, :], in0=sselP[:, 0:16, :], in1=sselP[:, 16:32, :], op=Alu.add)
                s2 = p16.tile([P, 8, g], F16, tag="c2")
                nc.vector.tensor_tensor(out=s2[:, :, :], in0=s1[:, 0:8, :], in1=s1[:, 8:16, :], op=Alu.add)
                s3 = p16.tile([P, 4, g], F16, tag="c3")
                nc.vector.tensor_tensor(out=s3[:, :, :], in0=s2[:, 0:4, :], in1=s2[:, 4:8, :], op=Alu.add)
                s4 = p16.tile([P, 2, g], F16, tag="c4")
                nc.vector.tensor_tensor(out=s4[:, :, :], in0=s3[:, 0:2, :], in1=s3[:, 2:4, :], op=Alu.add)
                s5 = p16.tile([P, 1, g], F16, tag="cnt2")
                nc.vector.tensor_tensor(out=s5[:, :, :], in0=s4[:, 0:1, :], in1=s4[:, 1:2, :], op=Alu.add)
                # ssel = s5 + EPS + sselP[K]  (fused; ss_t then holds final ssel)
                nc.vector.scalar_tensor_tensor(out=ss_t[:, t, :], in0=s5[:, 0, :],
                                               scalar=EPS, in1=sselP[:, K, :],
                                               op0=Alu.add, op1=Alu.add)

                # ---- W chain (fp16) ----
                dS = p16.tile([P, K, g], F16, tag="dS")
                nc.vector.tensor_tensor(out=dS[:, :, :], in0=Sh_t[:, 1:KP1, :], in1=Sh_t[:, 0:K, :], op=Alu.subtract)
                m_t = p16.tile([P, K, g], F16, tag="m")
                nc.vector.tensor_tensor(out=m_t[:, :, :], in0=M_t[:, :, :], in1=dS[:, :, :], op=Alu.mult)
                w_t = p16.tile([P, K, g], F16, tag="delta")
                nc.vector.tensor_tensor(out=w_t[:, :, :], in0=m_t[:, :, :], in1=xs_t[:, :, :], op=Alu.mult)
                t1 = p16.tile([P, 16, g], F16, tag="c1")
                nc.vector.tensor_tensor(out=t1[:, :, :], in0=w_t[:, 0:16, :], in1=w_t[:, 16:32, :], op=Alu.add)
                t2 = p16.tile([P, 8, g], F16, tag="c2")
                nc.vector.tensor_tensor(out=t2[:, :, :], in0=t1[:, 0:8, :], in1=t1[:, 8:16, :], op=Alu.add)
                t3 = p16.tile([P, 4, g], F16, tag="c3")
                nc.vector.tensor_tensor(out=t3[:, :, :], in0=t2[:, 0:4, :], in1=t2[:, 4:8, :], op=Alu.add)
                t4 = p16.tile([P, 2, g], F16, tag="c4")
                nc.vector.tensor_tensor(out=t4[:, :, :], in0=t3[:, 0:2, :], in1=t3[:, 2:4, :], op=Alu.add)
                nc.vector.tensor_tensor(out=W_t[:, t, :], in0=t4[:, 0, :], in1=t4[:, 1, :], op=Alu.add)

                # ---- v = xs0 * (S0 + EPS - 1)  (fused epilogue input) ----
                nc.vector.scalar_tensor_tensor(out=v_t[:, t, :], in0=Sh_t[:, 0, :],
                                               scalar=EPS - 1.0, in1=xs_t[:, 0, :],
                                               op0=Alu.add, op1=Alu.mult)

                if t == T // 2:
                    epilogue(0, T // 2)
            epilogue(T // 2, T)
    nc.compile()
    return nc


_NC_CACHE = {}


def _get_nc(nloc, g=G):
    key = (nloc, g)
    if key not in _NC_CACHE:
        _NC_CACHE[key] = build_nc(nloc, g)
    return _NC_CACHE[key]


def prepare_in_maps(inputs, x_pos, slope, y_bias):
    inputs = np.asarray(inputs, dtype=np.float32)
    x_pos = np.asarray(x_pos, dtype=np.float32)
    slope = np.asarray(slope, dtype=np.float32)
    y_bias = np.asarray(y_bias, dtype=np.float32)
    b, f = inputs.shape
    bloc = b // NCORES
    nloc = bloc * f
    T = nloc // (P * G)

    iota = np.arange(KP1, dtype=np.float16)
    iota_full = np.ascontiguousarray(
        np.broadcast_to(iota[None, :, None], (P, KP1, G)).reshape(P, KP1 * G)
    )
    pg_idx = (np.arange(P)[:, None] * G + np.arange(G)[None, :]) % f
    yb_exp = np.ascontiguousarray(y_bias[:, 0][pg_idx])

    in_maps = []
    for c in range(NCORES):
        sl_b = slice(c * bloc, (c + 1) * bloc)
        x = x_pos[sl_b].reshape(T, P, G, K).transpose(0, 1, 3, 2)
        x = np.ascontiguousarray(x).reshape(T * P, K * G)
        sl = slope[sl_b].astype(np.float16).reshape(T, P, G, KP1).transpose(0, 1, 3, 2)
        sl = np.ascontiguousarray(sl).reshape(T * P, KP1 * G)
        q = inputs[sl_b].reshape(T, P, G).transpose(1, 0, 2)
        q = np.ascontiguousarray(q).reshape(P, T * G)
        in_maps.append({"x": x, "sl": sl, "q": q, "iota": iota_full, "yb": yb_exp})
    return nloc, in_maps


def unpack_results(res, b, f):
    bloc = b // NCORES
    nloc = bloc * f
    T = nloc // (P * G)
    outs, ssels = [], []
    for c in range(NCORES):
        o = res.results[c]["out"].reshape(P, T, G).transpose(1, 0, 2).reshape(bloc, f)
        s = res.results[c]["ssel"].reshape(P, T, G).transpose(1, 0, 2).reshape(bloc, f)
        outs.append(o)
        ssels.append(s)
    return np.concatenate(outs, 0), np.concatenate(ssels, 0)


def kernel(inputs, x_pos, slope, y_bias):
    b, f = np.asarray(inputs).shape
    nloc, in_maps = prepare_in_maps(inputs, x_pos, slope, y_bias)
    nc = _get_nc(nloc)
    res = run_bass_kernel_spmd(nc, in_maps, list(range(NCORES)))
    return unpack_results(res, b, f)


# revision 13
# speedup vs baseline: 1.1351x; 1.0617x over previous
"""Trainium2 Bass kernel for nn_BlockPiecewiseLinear (histogram_binning).

Math (validated vs the JAX reference):
    S     = softplus(slope)  in fp16 (ACT: exp + ln(1+x))
    cnt   = sum_j 1[x_j <= q]  (UNSORTED x, fp32-exact compare)
    ssel  = S[cnt] + EPS   via  sum_j 1[cnt==j] * S_j   (one-hot: exact mask,
            only fp16 value rounding)
    xs    = sort(x) in fp16 (bitonic, 15 layers, all unit-stride -> DVE 2x)
    M_j   = 1[cnt >= j]
    W     = sum_j M_j * (S_j - S_{j-1}) * xs[j-1]        (fp16 chain)
    out   = q*ssel - xs[0]*(S_0+EPS-1) - W + y_bias

All compute on DVE+ACT.  GPSIMD is used only as a third DMA queue: running
Q7 compute concurrently with DVE steals the shared SBUF port and slows DVE
~2x (measured), so no compute is placed there.
Layout [P, K, G]: knot dim middle, G=128 rows innermost -> every op
unit-stride (fp16 2x mode); cnt path reads x in fp32 for exactness.
Sharding: pure data-parallel over batch across 8 NeuronCores.
"""

import numpy as np

import concourse.bass as bass
import concourse.bacc as bacc
import concourse.mybir as mybir
import concourse.tile as tile
from concourse.bass_utils import run_bass_kernel_spmd

F32 = mybir.dt.float32
F16 = mybir.dt.float16
Alu = mybir.AluOpType
Act = mybir.ActivationFunctionType
AxX = mybir.AxisListType.X

B, F, K = 4096, 512, 32
KP1 = K + 1
EPS = 1e-3
NCORES = 8
P = 128
G = 128  # rows (innermost) per tile per partition


def _bitonic_layers(n=32):
    layers = []
    k = 2
    while k <= n:
        j = k // 2
        while j >= 1:
            layers.append((k, j))
            j //= 2
        k *= 2
    return layers  # 15 layers for n=32


def _ap(t_ap, off_elems, dims):
    # count-1 free dims disqualify the DVE 2x perf mode; drop them
    dims = [e for e in dims if e[1] != 1]
    return bass.AP(tensor=t_ap.tensor, offset=t_ap.offset + off_elems,
                   ap=[t_ap.ap[0]] + dims)


def _emit_sort_layer(nc, cur, dst, kk, jj, g):
    """One bitonic layer in [P, K, G] layout (knot middle, rows inner)."""
    cs = K // (2 * kk) if kk < K else 0
    base_s = cur[:, :, :]
    base_d = dst[:, :, :]
    inner = [1, jj * g]
    if kk < K and jj == kk // 2:
        dims_in = [[2 * kk * g, cs], [kk * g, 2], inner]
        in_lo = _ap(base_s, 0, dims_in)
        in_hi = _ap(base_s, jj * g, dims_in)
        out_min = _ap(base_d, 0, [[2 * kk * g, cs], [(kk + jj) * g, 2], inner])
        out_max = _ap(base_d, jj * g, [[2 * kk * g, cs], [(kk - jj) * g, 2], inner])
        nc.vector.tensor_tensor(out=out_min, in0=in_lo, in1=in_hi, op=Alu.min)
        nc.vector.tensor_tensor(out=out_max, in0=in_lo, in1=in_hi, op=Alu.max)
    elif kk == K:
        ms = K // (2 * jj)
        dims_in = [[2 * jj * g, ms], inner]
        in_lo = _ap(base_s, 0, dims_in)
        in_hi = _ap(base_s, jj * g, dims_in)
        nc.vector.tensor_tensor(out=_ap(base_d, 0, dims_in), in0=in_lo, in1=in_hi, op=Alu.min)
        nc.vector.tensor_tensor(out=_ap(base_d, jj * g, dims_in), in0=in_lo, in1=in_hi, op=Alu.max)
    elif cs == 1:
        ms = kk // (2 * jj)
        dims_in = [[kk * g, 2], [2 * jj * g, ms], inner]
        in_lo = _ap(base_s, 0, dims_in)
        in_hi = _ap(base_s, jj * g, dims_in)
        out_min = _ap(base_d, 0, [[(kk + jj) * g, 2], [2 * jj * g, ms], inner])
        out_max = _ap(base_d, jj * g, [[(kk - jj) * g, 2], [2 * jj * g, ms], inner])
        nc.vector.tensor_tensor(out=out_min, in0=in_lo, in1=in_hi, op=Alu.min)
        nc.vector.tensor_tensor(out=out_max, in0=in_lo, in1=in_hi, op=Alu.max)
    else:
        ms = kk // (2 * jj)
        for d in range(2):
            off = d * kk * g
            dims_in = [[2 * kk * g, cs], [2 * jj * g, ms], inner]
            in_lo = _ap(base_s, off, dims_in)
            in_hi = _ap(base_s, off + jj * g, dims_in)
            o_lo = _ap(base_d, off, dims_in)
            o_hi = _ap(base_d, off + jj * g, dims_in)
            op_lo, op_hi = (Alu.min, Alu.max) if d == 0 else (Alu.max, Alu.min)
            nc.vector.tensor_tensor(out=o_lo, in0=in_lo, in1=in_hi, op=op_lo)
            nc.vector.tensor_tensor(out=o_hi, in0=in_lo, in1=in_hi, op=op_hi)


def build_nc(nloc, g=G):
    rows_per_tile = P * g
    T = nloc // rows_per_tile
    assert T * rows_per_tile == nloc
    TG = T * g

    nc = bacc.Bacc("TRN2", target_bir_lowering=False, debug=False)
    x_d = nc.declare_dram_parameter("x", [T * P, K * g], F32, isOutput=False)
    sl_d = nc.declare_dram_parameter("sl", [T * P, KP1 * g], F16, isOutput=False)
    q_d = nc.declare_dram_parameter("q", [P, TG], F32, isOutput=False)
    io_d = nc.declare_dram_parameter("iota", [P, KP1 * g], F16, isOutput=False)
    yb_d = nc.declare_dram_parameter("yb", [P, g], F32, isOutput=False)
    out_d = nc.declare_dram_parameter("out", [P, TG], F32, isOutput=True)
    ss_d = nc.declare_dram_parameter("ssel", [P, TG], F32, isOutput=True)

    xv = x_d[:, :].rearrange("(t p) f -> t p f", p=P)
    slv = sl_d[:, :].rearrange("(t p) f -> t p f", p=P)

    layers = _bitonic_layers(K)

    with tile.TileContext(nc) as tc:
        with (
            tc.tile_pool(name="pacc", bufs=1) as pacc,
            tc.tile_pool(name="pxf", bufs=2) as pxf,
            tc.tile_pool(name="pxh", bufs=2) as pxh,
            tc.tile_pool(name="psort", bufs=2) as psort,
            tc.tile_pool(name="psl", bufs=2) as psl,
            tc.tile_pool(name="pSh", bufs=2) as pSh,
            tc.tile_pool(name="p16", bufs=1) as p16,
        ):
            q_t = pacc.tile([P, T, g], F32, tag="q")
            nc.gpsimd.dma_start(out=q_t[:, 0:T // 2, :], in_=q_d[:, 0:TG // 2])
            iota_t = pacc.tile([P, KP1, g], F16, tag="iota")
            nc.scalar.dma_start(out=iota_t[:, :, :], in_=io_d[:, :])
            yb_t = pacc.tile([P, g], F32, tag="yb")
            nc.scalar.dma_start(out=yb_t[:, :], in_=yb_d[:, :])
            W_t = pacc.tile([P, T, g], F32, tag="W")
            ss_t = pacc.tile([P, T, g], F32, tag="ss")
            v_t = pacc.tile([P, T, g], F32, tag="v")

            def epilogue(lo, hi):
                # in-place on q_t (its slices are consumed by the tiles' stepU
                # before each half runs); out = q*ssel - v - W + yb  where
                # v = xs0*(S0+EPS-1) and ssel already includes +EPS.
                def s(t3):
                    return t3[:, lo:hi, :]
                nc.vector.tensor_tensor(out=s(q_t), in0=s(q_t), in1=s(ss_t), op=Alu.mult)
                nc.vector.tensor_tensor(out=s(q_t), in0=s(q_t), in1=s(v_t), op=Alu.subtract)
                nc.vector.tensor_tensor(out=s(q_t), in0=s(q_t), in1=s(W_t), op=Alu.subtract)
                ybb = bass.AP(tensor=yb_t[:, :].tensor, offset=yb_t[:, :].offset,
                              ap=[yb_t[:, :].ap[0], [0, hi - lo], [1, g]])
                nc.vector.tensor_tensor(out=s(q_t), in0=s(q_t), in1=ybb, op=Alu.add)
                nc.scalar.dma_start(out=out_d[:, lo * g:hi * g], in_=q_t[:, lo:hi, :])
                nc.scalar.dma_start(out=ss_d[:, lo * g:hi * g], in_=ss_t[:, lo:hi, :])

            for t in range(T):
                x_f = pxf.tile([P, K, g], F32, tag="x")
                if t == 0:
                    # split the first x tile across both queues to halve ramp
                    half = K * g // 2
                    nc.sync.dma_start(out=x_f[:, 0:K // 2, :], in_=xv[0][:, 0:half])
                    nc.gpsimd.dma_start(out=x_f[:, K // 2:K, :], in_=xv[0][:, half:])
                elif t % 2 == 0:
                    nc.sync.dma_start(out=x_f[:, :, :], in_=xv[t])
                else:
                    nc.gpsimd.dma_start(out=x_f[:, :, :], in_=xv[t])
                sl_t = psl.tile([P, KP1, g], F16, tag="sl")
                nc.scalar.dma_start(out=sl_t[:, :, :], in_=slv[t])
                if t == 1:
                    nc.gpsimd.dma_start(out=q_t[:, T // 2:T, :], in_=q_d[:, TG // 2:TG])

                # ---- ACT: softplus -> fp16, x cast -> fp16 ----
                Sh_t = pSh.tile([P, KP1, g], F16, tag="Sh")
                nc.scalar.activation(out=sl_t[:, :, :], in_=sl_t[:, :, :], func=Act.Exp)
                nc.scalar.activation(out=Sh_t[:, :, :], in_=sl_t[:, :, :], func=Act.Ln, bias=1.0)
                xh_t = pxh.tile([P, K, g], F16, tag="xh")
                nc.scalar.activation(out=xh_t[:, :, :], in_=x_f[:, :, :], func=Act.Copy)

                # ---- cnt: fp32-exact compare + fp16 binary-tree count ----
                stepU = p16.tile([P, K, g], F16, tag="su")
                q2 = q_t[:, t, :]
                qb = bass.AP(tensor=q2.tensor, offset=q2.offset,
                             ap=[q2.ap[0], [0, K], [1, g]])
                nc.vector.tensor_tensor(out=stepU[:, :, :], in0=x_f[:, :, :], in1=qb, op=Alu.is_le)
                c1 = p16.tile([P, 16, g], F16, tag="c1")
                nc.vector.tensor_tensor(out=c1[:, :, :], in0=stepU[:, 0:16, :], in1=stepU[:, 16:32, :], op=Alu.add)
                c2 = p16.tile([P, 8, g], F16, tag="c2")
                nc.vector.tensor_tensor(out=c2[:, :, :], in0=c1[:, 0:8, :], in1=c1[:, 8:16, :], op=Alu.add)
                c3 = p16.tile([P, 4, g], F16, tag="c3")
                nc.vector.tensor_tensor(out=c3[:, :, :], in0=c2[:, 0:4, :], in1=c2[:, 4:8, :], op=Alu.add)
                c4 = p16.tile([P, 2, g], F16, tag="c4")
                nc.vector.tensor_tensor(out=c4[:, :, :], in0=c3[:, 0:2, :], in1=c3[:, 2:4, :], op=Alu.add)
                cntt = p16.tile([P, 1, g], F16, tag="cnt")
                nc.vector.tensor_tensor(out=cntt[:, :, :], in0=c4[:, 0:1, :], in1=c4[:, 1:2, :], op=Alu.add)
                cnt = cntt[:, 0, :]

                # ---- fp16 bitonic sort ----
                cur = xh_t
                for kk, jj in layers:
                    dst = psort.tile([P, K, g], F16, tag="sort")
                    _emit_sort_layer(nc, cur, dst, kk, jj, g)
                    cur = dst
                xs_t = cur

                # ---- mask from cnt (emitted after the sort so the iota DMA
                # is never on the tile-0 critical path) ----
                cnt2 = cnt
                cb_k = bass.AP(tensor=cnt2.tensor, offset=cnt2.offset,
                               ap=[cnt2.ap[0], [0, K], [1, g]])
                M_t = p16.tile([P, K, g], F16, tag="M")
                nc.vector.tensor_tensor(out=M_t[:, :, :], in0=cb_k, in1=iota_t[:, 1:KP1, :], op=Alu.is_ge)

                # ---- W / ssel chain: m = M*dS, w = m*xs; the m and w halves
                # share one [2K, g] buffer so each tree level is a single op.
                # ssel = S0 + EPS + sum(m)   (telescoping: S[cnt] - S[0])
                # W    = sum(w)
                dS = p16.tile([P, K, g], F16, tag="dS")
                nc.vector.tensor_tensor(out=dS[:, :, :], in0=Sh_t[:, 1:KP1, :], in1=Sh_t[:, 0:K, :], op=Alu.subtract)
                mw = p16.tile([P, 2 * K, g], F16, tag="mw")
                nc.vector.tensor_tensor(out=mw[:, 0:K, :], in0=M_t[:, :, :], in1=dS[:, :, :], op=Alu.mult)
                nc.vector.tensor_tensor(out=mw[:, K:2 * K, :], in0=mw[:, 0:K, :], in1=xs_t[:, :, :], op=Alu.mult)
                mwb = mw[:, :, :]
                u1 = p16.tile([P, 32, g], F16, tag="su")
                nc.vector.tensor_tensor(
                    out=u1[:, :, :],
                    in0=_ap(mwb, 0, [[K * g, 2], [1, 16 * g]]),
                    in1=_ap(mwb, 16 * g, [[K * g, 2], [1, 16 * g]]), op=Alu.add)
                u1b = u1[:, :, :]
                u2 = p16.tile([P, 16, g], F16, tag="c1")
                nc.vector.tensor_tensor(
                    out=u2[:, :, :],
                    in0=_ap(u1b, 0, [[16 * g, 2], [1, 8 * g]]),
                    in1=_ap(u1b, 8 * g, [[16 * g, 2], [1, 8 * g]]), op=Alu.add)
                u2b = u2[:, :, :]
                u3 = p16.tile([P, 8, g], F16, tag="c2")
                nc.vector.tensor_tensor(
                    out=u3[:, :, :],
                    in0=_ap(u2b, 0, [[8 * g, 2], [1, 4 * g]]),
                    in1=_ap(u2b, 4 * g, [[8 * g, 2], [1, 4 * g]]), op=Alu.add)
                u3b = u3[:, :, :]
                u4 = p16.tile([P, 4, g], F16, tag="c3")
                nc.vector.tensor_tensor(
                    out=u4[:, :, :],
                    in0=_ap(u3b, 0, [[4 * g, 2], [1, 2 * g]]),
                    in1=_ap(u3b, 2 * g, [[4 * g, 2], [1, 2 * g]]), op=Alu.add)
                u4b = u4[:, :, :]
                u5 = p16.tile([P, 2, g], F16, tag="c4")
                nc.vector.tensor_tensor(
                    out=u5[:, :, :],
                    in0=_ap(u4b, 0, [[2 * g, 2], [1, g]]),
                    in1=_ap(u4b, g, [[2 * g, 2], [1, g]]), op=Alu.add)
                # u5[0] = sum(m) = S[cnt]-S[0];  u5[1] = W
                nc.vector.scalar_tensor_tensor(out=ss_t[:, t, :], in0=Sh_t[:, 0, :],
                                               scalar=EPS, in1=u5[:, 0, :],
                                               op0=Alu.add, op1=Alu.add)
                nc.vector.tensor_scalar_add(W_t[:, t, :], u5[:, 1, :], 0.0)

                # ---- v = xs0 * (S0 + EPS - 1)  (fused epilogue input) ----
                nc.vector.scalar_tensor_tensor(out=v_t[:, t, :], in0=Sh_t[:, 0, :],
                                               scalar=EPS - 1.0, in1=xs_t[:, 0, :],
                                               op0=Alu.add, op1=Alu.mult)

                if t == T // 2:
                    epilogue(0, T // 2)
            epilogue(T // 2, T)
    nc.compile()
    return nc


_NC_CACHE = {}


def _get_nc(nloc, g=G):
    key = (nloc, g)
    if key not in _NC_CACHE:
        _NC_CACHE[key] = build_nc(nloc, g)
    return _NC_CACHE[key]


def prepare_in_maps(inputs, x_pos, slope, y_bias):
    inputs = np.asarray(inputs, dtype=np.float32)
    x_pos = np.asarray(x_pos, dtype=np.float32)
    slope = np.asarray(slope, dtype=np.float32)
    y_bias = np.asarray(y_bias, dtype=np.float32)
    b, f = inputs.shape
    bloc = b // NCORES
    nloc = bloc * f
    T = nloc // (P * G)

    iota = np.arange(KP1, dtype=np.float16)
    iota_full = np.ascontiguousarray(
        np.broadcast_to(iota[None, :, None], (P, KP1, G)).reshape(P, KP1 * G)
    )
    pg_idx = (np.arange(P)[:, None] * G + np.arange(G)[None, :]) % f
    yb_exp = np.ascontiguousarray(y_bias[:, 0][pg_idx])

    in_maps = []
    for c in range(NCORES):
        sl_b = slice(c * bloc, (c + 1) * bloc)
        x = x_pos[sl_b].reshape(T, P, G, K).transpose(0, 1, 3, 2)
        x = np.ascontiguousarray(x).reshape(T * P, K * G)
        sl = slope[sl_b].astype(np.float16).reshape(T, P, G, KP1).transpose(0, 1, 3, 2)
        sl = np.ascontiguousarray(sl).reshape(T * P, KP1 * G)
        q = inputs[sl_b].reshape(T, P, G).transpose(1, 0, 2)
        q = np.ascontiguousarray(q).reshape(P, T * G)
        in_maps.append({"x": x, "sl": sl, "q": q, "iota": iota_full, "yb": yb_exp})
    return nloc, in_maps


def unpack_results(res, b, f):
    bloc = b // NCORES
    nloc = bloc * f
    T = nloc // (P * G)
    outs, ssels = [], []
    for c in range(NCORES):
        o = res.results[c]["out"].reshape(P, T, G).transpose(1, 0, 2).reshape(bloc, f)
        s = res.results[c]["ssel"].reshape(P, T, G).transpose(1, 0, 2).reshape(bloc, f)
        outs.append(o)
        ssels.append(s)
    return np.concatenate(outs, 0), np.concatenate(ssels, 0)


def kernel(inputs, x_pos, slope, y_bias):
    b, f = np.asarray(inputs).shape
    nloc, in_maps = prepare_in_maps(inputs, x_pos, slope, y_bias)
    nc = _get_nc(nloc)
    res = run_bass_kernel_spmd(nc, in_maps, list(range(NCORES)))
    return unpack_results(res, b, f)


# revision 16
# speedup vs baseline: 1.1632x; 1.0248x over previous
"""Trainium2 Bass kernel for nn_BlockPiecewiseLinear (histogram_binning).

Math (validated vs the JAX reference):
    S     = softplus(slope)  in fp16 (ACT: exp + ln(1+x))
    cnt   = sum_j 1[x_j <= q]  (UNSORTED x, fp32-exact compare)
    ssel  = S[cnt] + EPS   via  sum_j 1[cnt==j] * S_j   (one-hot: exact mask,
            only fp16 value rounding)
    xs    = sort(x) in fp16 (bitonic, 15 layers, all unit-stride -> DVE 2x)
    M_j   = 1[cnt >= j]
    W     = sum_j M_j * (S_j - S_{j-1}) * xs[j-1]        (fp16 chain)
    out   = q*ssel - xs[0]*(S_0+EPS-1) - W + y_bias

All compute on DVE+ACT.  GPSIMD is used only as a third DMA queue: running
Q7 compute concurrently with DVE steals the shared SBUF port and slows DVE
~2x (measured), so no compute is placed there.
Layout [P, K, G]: knot dim middle, G=128 rows innermost -> every op
unit-stride (fp16 2x mode); cnt path reads x in fp32 for exactness.
Sharding: pure data-parallel over batch across 8 NeuronCores.
"""

import numpy as np

import concourse.bass as bass
import concourse.bacc as bacc
import concourse.mybir as mybir
import concourse.tile as tile
from concourse.bass_utils import run_bass_kernel_spmd

F32 = mybir.dt.float32
F16 = mybir.dt.float16
Alu = mybir.AluOpType
Act = mybir.ActivationFunctionType
AxX = mybir.AxisListType.X

B, F, K = 4096, 512, 32
KP1 = K + 1
EPS = 1e-3
NCORES = 8
P = 128
G = 128  # rows (innermost) per tile per partition


def _bitonic_layers(n=32):
    layers = []
    k = 2
    while k <= n:
        j = k // 2
        while j >= 1:
            layers.append((k, j))
            j //= 2
        k *= 2
    return layers  # 15 layers for n=32


def _ap(t_ap, off_elems, dims):
    # count-1 free dims disqualify the DVE 2x perf mode; drop them
    dims = [e for e in dims if e[1] != 1]
    return bass.AP(tensor=t_ap.tensor, offset=t_ap.offset + off_elems,
                   ap=[t_ap.ap[0]] + dims)


def _emit_sort_layer(nc, cur, dst, kk, jj, g, npair=1):
    """One bitonic layer in [P, (pair,) K, G] layout (knot middle, rows inner).

    With npair=2 two tiles sit back-to-back (pair stride K*G).  Whenever an
    AP's outermost dim spans the full K*G densely, the pair dim coalesces
    into it (count x2) so both tiles are processed by ONE instruction.  The
    kk=16 middle sublayers' outputs can't (4 levels needed) and are emitted
    per pair member.
    """
    cs = K // (2 * kk) if kk < K else 0
    base_s = cur[:, :, :] if npair == 1 else cur[:, :, :, :]
    base_d = dst[:, :, :] if npair == 1 else dst[:, :, :, :]
    inner = [1, jj * g]

    def pair2(dims):
        # outermost spans K*g densely -> fold the pair dim into its count
        d0 = dims[0]
        assert d0[0] * d0[1] == K * g
        return [[d0[0], d0[1] * npair]] + dims[1:]

    if kk < K and jj == kk // 2:
        dims_in = pair2([[2 * kk * g, cs], [kk * g, 2], inner])
        in_lo = _ap(base_s, 0, dims_in)
        in_hi = _ap(base_s, jj * g, dims_in)
        out_min = _ap(base_d, 0, pair2([[2 * kk * g, cs], [(kk + jj) * g, 2], inner]))
        out_max = _ap(base_d, jj * g, pair2([[2 * kk * g, cs], [(kk - jj) * g, 2], inner]))
        nc.vector.tensor_tensor(out=out_min, in0=in_lo, in1=in_hi, op=Alu.min)
        nc.vector.tensor_tensor(out=out_max, in0=in_lo, in1=in_hi, op=Alu.max)
    elif kk == K:
        ms = K // (2 * jj)
        dims_in = pair2([[2 * jj * g, ms], inner])
        in_lo = _ap(base_s, 0, dims_in)
        in_hi = _ap(base_s, jj * g, dims_in)
        nc.vector.tensor_tensor(out=_ap(base_d, 0, dims_in), in0=in_lo, in1=in_hi, op=Alu.min)
        nc.vector.tensor_tensor(out=_ap(base_d, jj * g, dims_in), in0=in_lo, in1=in_hi, op=Alu.max)
    elif cs == 1:
        # kk=16 middle sublayers: min/max outputs interleave by direction and
        # cannot absorb the pair dim -> emit per pair member
        ms = kk // (2 * jj)
        for p in range(npair):
            off = p * K * g
            dims_in = [[kk * g, 2], [2 * jj * g, ms], inner]
            in_lo = _ap(base_s, off, dims_in)
            in_hi = _ap(base_s, off + jj * g, dims_in)
            out_min = _ap(base_d, off, [[(kk + jj) * g, 2], [2 * jj * g, ms], inner])
            out_max = _ap(base_d, off + jj * g, [[(kk - jj) * g, 2], [2 * jj * g, ms], inner])
            nc.vector.tensor_tensor(out=out_min, in0=in_lo, in1=in_hi, op=Alu.min)
            nc.vector.tensor_tensor(out=out_max, in0=in_lo, in1=in_hi, op=Alu.max)
    else:
        ms = kk // (2 * jj)
        for d in range(2):
            off = d * kk * g
            dims_in = pair2([[2 * kk * g, cs], [2 * jj * g, ms], inner])
            in_lo = _ap(base_s, off, dims_in)
            in_hi = _ap(base_s, off + jj * g, dims_in)
            o_lo = _ap(base_d, off, dims_in)
            o_hi = _ap(base_d, off + jj * g, dims_in)
            op_lo, op_hi = (Alu.min, Alu.max) if d == 0 else (Alu.max, Alu.min)
            nc.vector.tensor_tensor(out=o_lo, in0=in_lo, in1=in_hi, op=op_lo)
            nc.vector.tensor_tensor(out=o_hi, in0=in_lo, in1=in_hi, op=op_hi)


def build_nc(nloc, g=G):
    rows_per_tile = P * g
    T = nloc // rows_per_tile
    assert T * rows_per_tile == nloc
    TG = T * g

    nc = bacc.Bacc("TRN2", target_bir_lowering=False, debug=False)
    x_d = nc.declare_dram_parameter("x", [T * P, K * g], F32, isOutput=False)
    sl_d = nc.declare_dram_parameter("sl", [T * P, KP1 * g], F16, isOutput=False)
    q_d = nc.declare_dram_parameter("q", [P, TG], F32, isOutput=False)
    io_d = nc.declare_dram_parameter("iota", [P, KP1 * g], F16, isOutput=False)
    yb_d = nc.declare_dram_parameter("yb", [P, g], F32, isOutput=False)
    out_d = nc.declare_dram_parameter("out", [P, TG], F32, isOutput=True)
    ss_d = nc.declare_dram_parameter("ssel", [P, TG], F32, isOutput=True)

    xv = x_d[:, :].rearrange("(t p) f -> t p f", p=P)
    slv = sl_d[:, :].rearrange("(t p) f -> t p f", p=P)

    layers = _bitonic_layers(K)

    with tile.TileContext(nc) as tc:
        with (
            tc.tile_pool(name="pacc", bufs=1) as pacc,
            tc.tile_pool(name="pxf", bufs=2) as pxf,
            tc.tile_pool(name="pxh", bufs=2) as pxh,
            tc.tile_pool(name="psort", bufs=2) as psort,
            tc.tile_pool(name="psl", bufs=2) as psl,
            tc.tile_pool(name="pSh", bufs=2) as pSh,
            tc.tile_pool(name="p16", bufs=1) as p16,
        ):
            q_t = pacc.tile([P, T, g], F32, tag="q")
            nc.gpsimd.dma_start(out=q_t[:, 0:T // 2, :], in_=q_d[:, 0:TG // 2])
            iota_t = pacc.tile([P, KP1, g], F16, tag="iota")
            nc.scalar.dma_start(out=iota_t[:, :, :], in_=io_d[:, :])
            yb_t = pacc.tile([P, g], F32, tag="yb")
            nc.scalar.dma_start(out=yb_t[:, :], in_=yb_d[:, :])
            W_t = pacc.tile([P, T, g], F32, tag="W")
            ss_t = pacc.tile([P, T, g], F32, tag="ss")
            v_t = pacc.tile([P, T, g], F32, tag="v")

            def epilogue(lo, hi):
                # in-place on q_t (its slices are consumed by the tiles' stepU
                # before each half runs); out = q*ssel - v - W + yb  where
                # v = xs0*(S0+EPS-1) and ssel already includes +EPS.
                def s(t3):
                    return t3[:, lo:hi, :]
                nc.vector.tensor_tensor(out=s(q_t), in0=s(q_t), in1=s(ss_t), op=Alu.mult)
                nc.vector.tensor_tensor(out=s(q_t), in0=s(q_t), in1=s(v_t), op=Alu.subtract)
                nc.vector.tensor_tensor(out=s(q_t), in0=s(q_t), in1=s(W_t), op=Alu.subtract)
                ybb = bass.AP(tensor=yb_t[:, :].tensor, offset=yb_t[:, :].offset,
                              ap=[yb_t[:, :].ap[0], [0, hi - lo], [1, g]])
                nc.vector.tensor_tensor(out=s(q_t), in0=s(q_t), in1=ybb, op=Alu.add)
                nc.scalar.dma_start(out=out_d[:, lo * g:hi * g], in_=q_t[:, lo:hi, :])
                nc.scalar.dma_start(out=ss_d[:, lo * g:hi * g], in_=ss_t[:, lo:hi, :])

            for pt in range(T // 2):
                pair_xh = pxh.tile([P, 2, K, g], F16, tag="xh")
                pair_x = []
                pair_sh = []
                pair_cnt = []
                for i in (0, 1):
                    t = 2 * pt + i
                    x_f = pxf.tile([P, K, g], F32, tag="x")
                    pair_x.append(x_f)
                    if t == 0:
                        # split the first x tile across both queues (ramp)
                        half = K * g // 2
                        nc.sync.dma_start(out=x_f[:, 0:K // 2, :], in_=xv[0][:, 0:half])
                        nc.gpsimd.dma_start(out=x_f[:, K // 2:K, :], in_=xv[0][:, half:])
                    elif i == 0:
                        nc.sync.dma_start(out=x_f[:, :, :], in_=xv[t])
                    else:
                        nc.gpsimd.dma_start(out=x_f[:, :, :], in_=xv[t])
                    sl_t = psl.tile([P, KP1, g], F16, tag="sl")
                    nc.scalar.dma_start(out=sl_t[:, :, :], in_=slv[t])
                    if t == 1:
                        nc.gpsimd.dma_start(out=q_t[:, T // 2:T, :], in_=q_d[:, TG // 2:TG])

                    # ---- ACT: softplus -> fp16, x cast -> fp16 ----
                    Sh_t = pSh.tile([P, KP1, g], F16, tag="Sh")
                    nc.scalar.activation(out=sl_t[:, :, :], in_=sl_t[:, :, :], func=Act.Exp)
                    nc.scalar.activation(out=Sh_t[:, :, :], in_=sl_t[:, :, :], func=Act.Ln, bias=1.0)
                    pair_sh.append(Sh_t)
                    nc.scalar.activation(out=pair_xh[:, i, :, :], in_=x_f[:, :, :], func=Act.Copy)

                    # ---- cnt: fp32-exact compare + fp16 binary-tree count ----
                    stepU = p16.tile([P, K, g], F16, tag="su")
                    q2 = q_t[:, t, :]
                    qb = bass.AP(tensor=q2.tensor, offset=q2.offset,
                                 ap=[q2.ap[0], [0, K], [1, g]])
                    nc.vector.tensor_tensor(out=stepU[:, :, :], in0=x_f[:, :, :], in1=qb, op=Alu.is_le)
                    c1 = p16.tile([P, 16, g], F16, tag="c1")
                    nc.vector.tensor_tensor(out=c1[:, :, :], in0=stepU[:, 0:16, :], in1=stepU[:, 16:32, :], op=Alu.add)
                    c2 = p16.tile([P, 8, g], F16, tag="c2")
                    nc.vector.tensor_tensor(out=c2[:, :, :], in0=c1[:, 0:8, :], in1=c1[:, 8:16, :], op=Alu.add)
                    c3 = p16.tile([P, 4, g], F16, tag="c3")
                    nc.vector.tensor_tensor(out=c3[:, :, :], in0=c2[:, 0:4, :], in1=c2[:, 4:8, :], op=Alu.add)
                    c4 = p16.tile([P, 2, g], F16, tag="c4")
                    nc.vector.tensor_tensor(out=c4[:, :, :], in0=c3[:, 0:2, :], in1=c3[:, 2:4, :], op=Alu.add)
                    cntt = p16.tile([P, 1, g], F16, tag=f"cnt{i}")
                    nc.vector.tensor_tensor(out=cntt[:, :, :], in0=c4[:, 0:1, :], in1=c4[:, 1:2, :], op=Alu.add)
                    pair_cnt.append(cntt[:, 0, :])

                # ---- fp16 bitonic sort over the tile pair ----
                cur = pair_xh
                for kk, jj in layers:
                    dst = psort.tile([P, 2, K, g], F16, tag="sort")
                    _emit_sort_layer(nc, cur, dst, kk, jj, g, npair=2)
                    cur = dst

                for i in (0, 1):
                    t = 2 * pt + i
                    xs_t = cur[:, i]
                    Sh_t = pair_sh[i]
                    cnt2 = pair_cnt[i]

                    # ---- mask from cnt; W/ssel chain in one [2K, g] buffer:
                    #   mw[K:2K] = dS,  mw[0:K] = M, then m = M*dS in place,
                    #   w = m*xs over dS's slot.  Shared tree sums both halves.
                    #   ssel = S0 + EPS + sum(m)   (telescoping S[cnt]-S[0])
                    cb_k = bass.AP(tensor=cnt2.tensor, offset=cnt2.offset,
                                   ap=[cnt2.ap[0], [0, K], [1, g]])
                    mw = p16.tile([P, 2 * K, g], F16, tag="mw")
                    nc.vector.tensor_tensor(out=mw[:, K:2 * K, :], in0=Sh_t[:, 1:KP1, :],
                                            in1=Sh_t[:, 0:K, :], op=Alu.subtract)
                    nc.vector.tensor_tensor(out=mw[:, 0:K, :], in0=cb_k,
                                            in1=iota_t[:, 1:KP1, :], op=Alu.is_ge)
                    nc.vector.tensor_tensor(out=mw[:, 0:K, :], in0=mw[:, 0:K, :],
                                            in1=mw[:, K:2 * K, :], op=Alu.mult)
                    nc.vector.tensor_tensor(out=mw[:, K:2 * K, :], in0=mw[:, 0:K, :],
                                            in1=xs_t[:, :, :], op=Alu.mult)
                    mwb = mw[:, :, :]
                    u1 = p16.tile([P, 32, g], F16, tag="su")
                    nc.vector.tensor_tensor(
                        out=u1[:, :, :],
                        in0=_ap(mwb, 0, [[K * g, 2], [1, 16 * g]]),
                        in1=_ap(mwb, 16 * g, [[K * g, 2], [1, 16 * g]]), op=Alu.add)
                    u1b = u1[:, :, :]
                    u2 = p16.tile([P, 16, g], F16, tag="c1")
                    nc.vector.tensor_tensor(
                        out=u2[:, :, :],
                        in0=_ap(u1b, 0, [[16 * g, 2], [1, 8 * g]]),
                        in1=_ap(u1b, 8 * g, [[16 * g, 2], [1, 8 * g]]), op=Alu.add)
                    u2b = u2[:, :, :]
                    u3 = p16.tile([P, 8, g], F16, tag="c2")
                    nc.vector.tensor_tensor(
                        out=u3[:, :, :],
                        in0=_ap(u2b, 0, [[8 * g, 2], [1, 4 * g]]),
                        in1=_ap(u2b, 4 * g, [[8 * g, 2], [1, 4 * g]]), op=Alu.add)
                    u3b = u3[:, :, :]
                    u4 = p16.tile([P, 4, g], F16, tag="c3")
                    nc.vector.tensor_tensor(
                        out=u4[:, :, :],
                        in0=_ap(u3b, 0, [[4 * g, 2], [1, 2 * g]]),
                        in1=_ap(u3b, 2 * g, [[4 * g, 2], [1, 2 * g]]), op=Alu.add)
                    u4b = u4[:, :, :]
                    u5 = p16.tile([P, 2, g], F16, tag="c4")
                    nc.vector.tensor_tensor(
                        out=u5[:, :, :],
                        in0=_ap(u4b, 0, [[2 * g, 2], [1, g]]),
                        in1=_ap(u4b, g, [[2 * g, 2], [1, g]]), op=Alu.add)
                    # u5[0] = sum(m) = S[cnt]-S[0];  u5[1] = W
                    nc.vector.scalar_tensor_tensor(out=ss_t[:, t, :], in0=Sh_t[:, 0, :],
                                                   scalar=EPS, in1=u5[:, 0, :],
                                                   op0=Alu.add, op1=Alu.add)
                    nc.vector.tensor_scalar_add(W_t[:, t, :], u5[:, 1, :], 0.0)
                    # v = xs0 * (S0 + EPS - 1)  (fused epilogue input)
                    nc.vector.scalar_tensor_tensor(out=v_t[:, t, :], in0=Sh_t[:, 0, :],
                                                   scalar=EPS - 1.0, in1=xs_t[:, 0, :],
                                                   op0=Alu.add, op1=Alu.mult)

                if 2 * pt + 1 == T // 2 - 1:
                    epilogue(0, T // 2)
            epilogue(T // 2, T)
    nc.compile()
    return nc


_NC_CACHE = {}


def _get_nc(nloc, g=G):
    key = (nloc, g)
    if key not in _NC_CACHE:
        _NC_CACHE[key] = build_nc(nloc, g)
    return _NC_CACHE[key]


def prepare_in_maps(inputs, x_pos, slope, y_bias):
    inputs = np.asarray(inputs, dtype=np.float32)
    x_pos = np.asarray(x_pos, dtype=np.float32)
    slope = np.asarray(slope, dtype=np.float32)
    y_bias = np.asarray(y_bias, dtype=np.float32)
    b, f = inputs.shape
    bloc = b // NCORES
    nloc = bloc * f
    T = nloc // (P * G)

    iota = np.arange(KP1, dtype=np.float16)
    iota_full = np.ascontiguousarray(
        np.broadcast_to(iota[None, :, None], (P, KP1, G)).reshape(P, KP1 * G)
    )
    pg_idx = (np.arange(P)[:, None] * G + np.arange(G)[None, :]) % f
    yb_exp = np.ascontiguousarray(y_bias[:, 0][pg_idx])

    in_maps = []
    for c in range(NCORES):
        sl_b = slice(c * bloc, (c + 1) * bloc)
        x = x_pos[sl_b].reshape(T, P, G, K).transpose(0, 1, 3, 2)
        x = np.ascontiguousarray(x).reshape(T * P, K * G)
        sl = slope[sl_b].astype(np.float16).reshape(T, P, G, KP1).transpose(0, 1, 3, 2)
        sl = np.ascontiguousarray(sl).reshape(T * P, KP1 * G)
        q = inputs[sl_b].reshape(T, P, G).transpose(1, 0, 2)
        q = np.ascontiguousarray(q).reshape(P, T * G)
        in_maps.append({"x": x, "sl": sl, "q": q, "iota": iota_full, "yb": yb_exp})
    return nloc, in_maps


def unpack_results(res, b, f):
    bloc = b // NCORES
    nloc = bloc * f
    T = nloc // (P * G)
    outs, ssels = [], []
    for c in range(NCORES):
        o = res.results[c]["out"].reshape(P, T, G).transpose(1, 0, 2).reshape(bloc, f)
        s = res.results[c]["ssel"].reshape(P, T, G).transpose(1, 0, 2).reshape(bloc, f)
        outs.append(o)
        ssels.append(s)
    return np.concatenate(outs, 0), np.concatenate(ssels, 0)


def kernel(inputs, x_pos, slope, y_bias):
    b, f = np.asarray(inputs).shape
    nloc, in_maps = prepare_in_maps(inputs, x_pos, slope, y_bias)
    nc = _get_nc(nloc)
    res = run_bass_kernel_spmd(nc, in_maps, list(range(NCORES)))
    return unpack_results(res, b, f)
